# revision 23
# baseline (speedup 1.0000x reference)
"""Trainium2 Bass kernel for nn_NodeBlock (GNN message passing).

Computes, for a graph with N=100000 nodes and E=1600000 edges:
    agg = segment_sum(edge_attr, edge_index[1], N)        # [N, 64]
    h   = relu(concat([x, agg], 1) @ W1 + b1)             # [N, 256]
    out = h @ W2 + b2                                     # [N, 64]

Strategy (8 NeuronCores, no collectives):
  * Nodes are packed into blocks of <= NODE_CAP=32 nodes such that every
    block receives <= T_MAX*128 = 512 edges (host-side serpentine packing
    on in-degree).  Core c owns blocks [c*400, (c+1)*400).
  * Edges are bucketed by receiver block on the host, so each core's
    shard holds exactly the edges targeting its own nodes: no all-reduce.
  * fp32 values are shipped as bf16 (hi, lo) pairs (hi = bf16(v),
    lo = bf16(v - hi)); matmuls run at bf16 speed and all accumulation is
    fp32 in PSUM, keeping ~2^-18 relative precision.
  * Scatter-add: each 128-edge tile becomes a one-hot [128, 32] matrix
    (iota vs local-node-index compare, built 32 tiles per vector op) and
    a single matmul with the fused [hi|lo] stationary accumulates
    [hi-feat; lo-feat] partial aggregates [128, 32] in PSUM.  The hi/lo
    fold happens for free inside the MLP's agg matmul, whose fp32 weight
    is W1a stacked twice along the contraction dim.
  * The MLP runs on 256-node groups in feature-major layout (nodes on
    the free dim); the x-side and W2-side matmuls use bf16 (hi, lo)
    3-term products, with cross terms fused by stacking [lo; hi] inputs
    against [hi; lo] weights along the contraction dim.
"""

import sys

sys.path.insert(0, "/opt/trn_rl_repo")

import numpy as np

# ---------------------------------------------------------------- constants
N_NODES = 100000
N_EDGES = 1600000
D = 64            # d_node == d_edge == d_out
D_HID = 256
N_CORES = 8

NODE_CAP = 32         # node slots per block
N_BINS = 3200         # total blocks (divisible by N_CORES)
BLOCKS_PER_CORE = N_BINS // N_CORES          # 400
GROUP_BLOCKS = 8                             # blocks per MLP group
SLOTS = N_BINS * NODE_CAP                    # 102400 padded node slots
SLOTS_PER_CORE = SLOTS // N_CORES            # 12800

_cache = {}


def _split(a, bf16):
    """Return (hi, lo) bf16 pair with hi + lo ~= a (to ~2^-18 rel)."""
    hi = a.astype(bf16)
    lo = (a - hi.astype(np.float32)).astype(bf16)
    return hi, lo


# ---------------------------------------------------------------- packing
def _pack_nodes(deg):
    """Assign each node to a (block, slot) so that every block has at most
    NODE_CAP nodes and block in-degree sums are nearly equal (serpentine on
    sorted degree).  Returns (orig, inv): orig[slot] = node id or -1,
    inv[node] = slot."""
    order = np.argsort(-deg, kind="stable")
    n = order.size
    rows = np.arange(n) // N_BINS
    cols = np.arange(n) % N_BINS
    cols = np.where(rows % 2 == 0, cols, N_BINS - 1 - cols)
    slot = cols * NODE_CAP + rows
    orig = np.full(SLOTS, -1, dtype=np.int64)
    orig[slot] = order
    inv = np.empty(n, dtype=np.int64)
    inv[order] = slot
    return orig, inv


def _preprocess(x, edge_attr, receivers):
    """Build per-core device arrays.  Returns (in_maps, orig, t_max)."""
    import ml_dtypes

    bf16 = ml_dtypes.bfloat16
    deg = np.bincount(receivers, minlength=N_NODES)
    orig, inv = _pack_nodes(deg)

    eslot = inv[receivers]                  # node slot of each edge's receiver
    blk = eslot // NODE_CAP                 # block id per edge
    lidx = (eslot % NODE_CAP).astype(np.float32)

    counts = np.bincount(blk, minlength=N_BINS)
    t_max = max(4, int(-(-counts.max() // 128)))   # tiles (of 128 edges) per block

    order_e = np.argsort(blk, kind="stable")
    blk_s = blk[order_e]
    starts = np.zeros(N_BINS, dtype=np.int64)
    np.cumsum(counts[:-1], out=starts[1:])
    pos = np.arange(N_EDGES) - starts[blk_s]       # position within block
    k = pos // 128
    p = pos % 128

    core = blk_s // BLOCKS_PER_CORE
    gblk = blk_s % BLOCKS_PER_CORE
    lidx_s = lidx[order_e]

    hi, lo = _split(edge_attr[order_e], bf16)

    # padded node features: x as bf16 (hi, lo), feature-major
    xt_full = np.zeros((SLOTS, D), dtype=np.float32)
    valid = orig >= 0
    xt_full[valid] = x[orig[valid]]
    xh_full, xl_full = _split(xt_full, bf16)

    n_tiles = BLOCKS_PER_CORE * t_max
    G = BLOCKS_PER_CORE // GROUP_BLOCKS
    in_maps = []
    for c in range(N_CORES):
        sel = core == c
        # layout: [G, 128 partitions, block-in-group, tile, {hi,lo}, 64]
        # -> each partition's slice of a group DMA is 8KB contiguous, and
        #    tile t's 128 columns are the fused [hi | lo] stationary.
        pay = np.zeros((G, 128, GROUP_BLOCKS, t_max, 2, D), dtype=bf16)
        g_ = gblk[sel] // GROUP_BLOCKS
        b_ = gblk[sel] % GROUP_BLOCKS
        pay[g_, p[sel], b_, k[sel], 0, :] = hi[sel]
        pay[g_, p[sel], b_, k[sel], 1, :] = lo[sel]
        la = np.full((128, n_tiles), float(NODE_CAP), dtype=bf16)
        la[p[sel], gblk[sel] * t_max + k[sel]] = lidx_s[sel]
        sl = slice(c * SLOTS_PER_CORE, (c + 1) * SLOTS_PER_CORE)
        xh = np.ascontiguousarray(xh_full[sl].T)             # [64, S] bf16
        xl = np.ascontiguousarray(xl_full[sl].T)             # [64, S] bf16
        in_maps.append(
            {
                "edges": pay.reshape(G * 128, GROUP_BLOCKS * t_max * 2 * D),
                "lidx": la,
                "xc": np.ascontiguousarray(np.vstack([xl, xh])),  # [128, S]
            }
        )
    return in_maps, orig, t_max


# ---------------------------------------------------------------- program
def _build_program(t_max):
    from contextlib import ExitStack

    import concourse.bacc as bacc
    import concourse.tile as tile
    from concourse import mybir

    f32 = mybir.dt.float32
    bf16 = mybir.dt.bfloat16
    G = BLOCKS_PER_CORE // GROUP_BLOCKS      # MLP groups per core (50)
    TPG = GROUP_BLOCKS * t_max               # edge tiles per group
    NPG = GROUP_BLOCKS * NODE_CAP            # nodes per group (256)
    NT = BLOCKS_PER_CORE * t_max             # edge tiles per core
    S = SLOTS_PER_CORE
    RELU = mybir.ActivationFunctionType.Relu

    nc = bacc.Bacc("TRN2", target_bir_lowering=False, debug=False)
    edges = nc.dram_tensor(
        "edges", [G * 128, TPG * 2 * D], bf16, kind="ExternalInput"
    ).ap()
    lidx = nc.dram_tensor("lidx", [128, NT], bf16, kind="ExternalInput").ap()
    xc = nc.dram_tensor("xc", [128, S], bf16, kind="ExternalInput").ap()
    w1xs = nc.dram_tensor("w1xs", [128, D_HID], bf16, kind="ExternalInput").ap()
    # W1x_hi lives in partitions 64:128 so it can contract directly against
    # the x_hi half of xc (same base partition); rows 0:64 are zero padding.
    w1xh = nc.dram_tensor("w1xh", [128, D_HID], bf16, kind="ExternalInput").ap()
    w1aa = nc.dram_tensor("w1aa", [128, D_HID], f32, kind="ExternalInput").ap()
    w2h = nc.dram_tensor("w2h", [128, 128], bf16, kind="ExternalInput").ap()
    w2l = nc.dram_tensor("w2l", [128, 128], bf16, kind="ExternalInput").ap()
    b1 = nc.dram_tensor("b1", [128, 2], f32, kind="ExternalInput").ap()
    b2 = nc.dram_tensor("b2", [D, 1], f32, kind="ExternalInput").ap()
    out = nc.dram_tensor("out_t", [D, S], f32, kind="ExternalOutput").ap()

    with tile.TileContext(nc) as tc, ExitStack() as ctx:
        const = ctx.enter_context(tc.tile_pool(name="const", bufs=1))
        epool = ctx.enter_context(tc.tile_pool(name="epool", bufs=4))
        ohpool = ctx.enter_context(tc.tile_pool(name="ohpool", bufs=3))
        spool = ctx.enter_context(tc.tile_pool(name="spool", bufs=3))
        hpool = ctx.enter_context(tc.tile_pool(name="hpool", bufs=4))
        ps_a = ctx.enter_context(tc.tile_pool(name="ps_a", bufs=2, space="PSUM"))
        ps_h = ctx.enter_context(tc.tile_pool(name="ps_h", bufs=4, space="PSUM"))
        ps_o = ctx.enter_context(tc.tile_pool(name="ps_o", bufs=2, space="PSUM"))

        iota_i = const.tile([128, TPG * NODE_CAP], mybir.dt.int32)
        nc.gpsimd.iota(
            iota_i[:], pattern=[[0, TPG], [1, NODE_CAP]], channel_multiplier=0
        )
        iota_b = const.tile([128, TPG * NODE_CAP], bf16)
        nc.vector.tensor_copy(iota_b[:], iota_i[:])

        xc_sb = const.tile([128, S], bf16)
        nc.sync.dma_start(xc_sb[:], xc[:])
        lidx_sb = const.tile([128, NT], bf16)
        nc.sync.dma_start(lidx_sb[:], lidx[:])
        w1xs_sb = const.tile([128, D_HID], bf16)
        nc.sync.dma_start(w1xs_sb[:], w1xs[:])
        w1xh_sb = const.tile([128, D_HID], bf16)
        nc.sync.dma_start(w1xh_sb[:], w1xh[:])
        w1aa_sb = const.tile([128, D_HID], f32)
        nc.sync.dma_start(w1aa_sb[:], w1aa[:])
        w2h_sb = const.tile([128, 128], bf16)
        nc.sync.dma_start(w2h_sb[:], w2h[:])
        w2l_sb = const.tile([128, 128], bf16)
        nc.sync.dma_start(w2l_sb[:], w2l[:])
        b1_sb = const.tile([128, 2], f32)
        nc.sync.dma_start(b1_sb[:], b1[:])
        b2_sb = const.tile([D, 1], f32)
        nc.sync.dma_start(b2_sb[:], b2[:])

        # output staged in chunks so the store DMA streams during compute
        OCHUNK = 10                                   # groups per out chunk
        out_tiles = [
            const.tile([D, OCHUNK * NPG], f32, name=f"outc{i}", tag=f"out{i}")
            for i in range((G + OCHUNK - 1) // OCHUNK)
        ]

        def scatter_stage(g):
            """DMA + one-hot + 32 scatter matmuls -> (agg PSUM, sums SBUF)."""
            ech = epool.tile([128, TPG * 2 * D], bf16)
            nc.sync.dma_start(ech[:], edges[g * 128 : (g + 1) * 128, :])
            oh = ohpool.tile([128, TPG * NODE_CAP], bf16)
            nc.vector.tensor_tensor(
                out=oh[:].rearrange("p (t l) -> p t l", l=NODE_CAP),
                in0=iota_b[:].rearrange("p (t l) -> p t l", l=NODE_CAP),
                in1=lidx_sb[:, g * TPG : (g + 1) * TPG, None].to_broadcast(
                    [128, TPG, NODE_CAP]
                ),
                op=mybir.AluOpType.is_equal,
            )
            # one matmul per 128-edge tile with the fused [hi|lo] stationary
            # -> PSUM rows 0:64 = sum(hi), rows 64:128 = sum(lo)
            agg_ps = ps_a.tile([128, NPG], f32)
            for b in range(GROUP_BLOCKS):
                for kk in range(t_max):
                    t = b * t_max + kk
                    nc.tensor.matmul(
                        out=agg_ps[:, b * NODE_CAP : (b + 1) * NODE_CAP],
                        lhsT=ech[:, t * 2 * D : (t + 1) * 2 * D],
                        rhs=oh[:, t * NODE_CAP : (t + 1) * NODE_CAP],
                        start=(kk == 0),
                        stop=(kk == t_max - 1),
                    )
            sums_sb = spool.tile([128, NPG], f32)
            nc.vector.tensor_copy(sums_sb[:], agg_ps[:])
            return sums_sb

        def h_stage(g, sums_sb):
            """6 matmuls + relu + bf16 (hi, lo) split of h."""
            gs = slice(g * NPG, (g + 1) * NPG)
            his, los = [], []
            for hh in range(2):
                hsl = slice(hh * 128, (hh + 1) * 128)
                h_ps = ps_h.tile([128, NPG], f32)
                # cross terms: [W1x_hi; W1x_lo] against [x_lo; x_hi]
                nc.tensor.matmul(
                    out=h_ps[:], lhsT=w1xs_sb[:, hsl], rhs=xc_sb[:, gs],
                    start=True, stop=False,
                )
                # main term: W1x_hi @ x_hi (both at base partition 64)
                nc.tensor.matmul(
                    out=h_ps[:], lhsT=w1xh_sb[D:, hsl], rhs=xc_sb[D:, gs],
                    start=False, stop=False,
                )
                # agg term (fp32, exact): [W1a; W1a] @ [sum_hi; sum_lo]
                nc.tensor.matmul(
                    out=h_ps[:], lhsT=w1aa_sb[:, hsl], rhs=sums_sb[:],
                    start=False, stop=True,
                )
                h_f = hpool.tile([128, NPG], f32, tag="h_f")
                nc.scalar.activation(
                    h_f[:], h_ps[:], RELU, bias=b1_sb[:, hh : hh + 1], scale=1.0
                )
                h_hi = hpool.tile([128, NPG], bf16, tag="h_hi")
                nc.vector.tensor_copy(h_hi[:], h_f[:])
                h_lo = hpool.tile([128, NPG], bf16, tag="h_lo")
                nc.gpsimd.tensor_tensor(
                    out=h_lo[:], in0=h_f[:], in1=h_hi[:],
                    op=mybir.AluOpType.subtract,
                )
                his.append(h_hi)
                los.append(h_lo)
            return his, los

        def out_stage(g, his, los):
            """6 bf16 matmuls + bias; stream the chunk DMA when complete."""
            o_ps = ps_o.tile([D, NPG], f32)
            mms = []
            for hh in range(2):
                csl = slice(hh * D, (hh + 1) * D)
                mms += [
                    (w2h_sb[:, csl], his[hh][:]),
                    (w2h_sb[:, csl], los[hh][:]),
                    (w2l_sb[:, csl], his[hh][:]),
                ]
            for i, (lt, rt) in enumerate(mms):
                nc.tensor.matmul(
                    out=o_ps[:], lhsT=lt, rhs=rt,
                    start=(i == 0), stop=(i == len(mms) - 1),
                )
            ci, co = g // OCHUNK, g % OCHUNK
            otile = out_tiles[ci]
            nc.vector.tensor_scalar(
                out=otile[:, co * NPG : (co + 1) * NPG],
                in0=o_ps[:],
                scalar1=b2_sb[:, 0:1],
                scalar2=None,
                op0=mybir.AluOpType.add,
            )
            lastco = min(OCHUNK, G - ci * OCHUNK) - 1
            if co == lastco:
                nc.sync.dma_start(
                    out[:, ci * OCHUNK * NPG : ci * OCHUNK * NPG + (lastco + 1) * NPG],
                    otile[:, : (lastco + 1) * NPG],
                )

        # software pipeline: each group's out-stage is emitted after the next
        # group's scatter, so the PE has independent work while the
        # relu/split chain (ACT -> DVE -> GpSimd) completes.
        pend = None
        for g in range(G):
            sums_sb = scatter_stage(g)
            if pend is not None:
                out_stage(*pend)
            pend = (g, *h_stage(g, sums_sb))
        out_stage(*pend)

    nc.compile()
    return nc


def _get_program(t_max):
    if t_max not in _cache:
        _cache[t_max] = _build_program(t_max)
    return _cache[t_max]


def _weight_maps(W1, b1, W2, b2):
    import ml_dtypes

    bf16 = ml_dtypes.bfloat16
    w1x = np.ascontiguousarray(W1[0:D, :])
    w1a = np.ascontiguousarray(W1[D : 2 * D, :])
    w1x_hi, w1x_lo = _split(w1x, bf16)
    w2p = np.concatenate([W2[0:128, :], W2[128:256, :]], axis=1)  # [128, 128]
    w2_hi, w2_lo = _split(w2p, bf16)
    return {
        "w1xs": np.ascontiguousarray(np.vstack([w1x_hi, w1x_lo])),
        "w1xh": np.ascontiguousarray(
            np.vstack([np.zeros_like(w1x_hi), w1x_hi])
        ),
        "w1aa": np.ascontiguousarray(np.vstack([w1a, w1a])),
        "w2h": np.ascontiguousarray(w2_hi),
        "w2l": np.ascontiguousarray(w2_lo),
        "b1": np.ascontiguousarray(b1.reshape(2, 128).T),
        "b2": np.ascontiguousarray(b2.reshape(D, 1)),
    }


# ---------------------------------------------------------------- entry
def kernel(x, edge_attr, edge_index, pos, W1, b1, W2, b2, _trace=False, _tmpdir=None):
    from concourse.bass_utils import run_bass_kernel_spmd

    x = np.asarray(x, dtype=np.float32)
    edge_attr = np.asarray(edge_attr, dtype=np.float32)
    receivers = np.asarray(edge_index[1]).astype(np.int64)
    W1 = np.asarray(W1, dtype=np.float32)
    b1 = np.asarray(b1, dtype=np.float32)
    W2 = np.asarray(W2, dtype=np.float32)
    b2 = np.asarray(b2, dtype=np.float32)

    in_maps, orig, t_max = _preprocess(x, edge_attr, receivers)
    wmap = _weight_maps(W1, b1, W2, b2)
    for m in in_maps:
        m.update(wmap)

    nc = _get_program(t_max)
    res = run_bass_kernel_spmd(
        nc, in_maps, list(range(N_CORES)), trace=_trace, tmpdir=_tmpdir
    )

    big = np.concatenate([r["out_t"] for r in res.results], axis=1)  # [64, SLOTS]
    valid = orig >= 0
    result = np.empty((N_NODES, D), dtype=np.float32)
    result[orig[valid]] = big.T[valid]
    if _trace:
        kernel.last_results = res
    return result


# revision 28
# speedup vs baseline: 1.3236x; 1.3236x over previous
"""Trainium2 Bass kernel for nn_NodeBlock (GNN message passing).

Computes, for a graph with N=100000 nodes and E=1600000 edges:
    agg = segment_sum(edge_attr, edge_index[1], N)        # [N, 64]
    h   = relu(concat([x, agg], 1) @ W1 + b1)             # [N, 256]
    out = h @ W2 + b2                                     # [N, 64]

Strategy (8 NeuronCores, no collectives):
  * Nodes are packed into blocks of <= NODE_CAP=32 nodes such that every
    block receives <= T_MAX*128 = 512 edges (host-side serpentine packing
    on in-degree).  Core c owns blocks [c*400, (c+1)*400).
  * Edges are bucketed by receiver block on the host, so each core's
    shard holds exactly the edges targeting its own nodes: no all-reduce.
  * fp32 values are shipped as bf16 (hi, lo) pairs (hi = bf16(v),
    lo = bf16(v - hi)); matmuls run at bf16 speed and all accumulation is
    fp32 in PSUM, keeping ~2^-18 relative precision.
  * Scatter-add: each 128-edge tile becomes a one-hot [128, 32] matrix
    (iota vs local-node-index compare, built 32 tiles per vector op) and
    a single matmul with the fused [hi|lo] stationary accumulates
    [hi-feat; lo-feat] partial aggregates [128, 32] in PSUM.  The hi/lo
    fold happens for free inside the MLP's agg matmul, whose fp32 weight
    is W1a stacked twice along the contraction dim.
  * The MLP runs on 256-node groups in feature-major layout (nodes on
    the free dim); the x-side and W2-side matmuls use bf16 (hi, lo)
    3-term products, with cross terms fused by stacking [lo; hi] inputs
    against [hi; lo] weights along the contraction dim.
"""

import sys

sys.path.insert(0, "/opt/trn_rl_repo")

import numpy as np

# ---------------------------------------------------------------- constants
N_NODES = 100000
N_EDGES = 1600000
D = 64            # d_node == d_edge == d_out
D_HID = 256
N_CORES = 8

NODE_CAP = 32         # node slots per block
N_BINS = 3200         # total blocks (divisible by N_CORES)
BLOCKS_PER_CORE = N_BINS // N_CORES          # 400
GROUP_BLOCKS = 16                            # blocks per MLP group
SLOTS = N_BINS * NODE_CAP                    # 102400 padded node slots
SLOTS_PER_CORE = SLOTS // N_CORES            # 12800

_cache = {}


def _split(a, bf16):
    """Return (hi, lo) bf16 pair with hi + lo ~= a (to ~2^-18 rel)."""
    hi = a.astype(bf16)
    lo = (a - hi.astype(np.float32)).astype(bf16)
    return hi, lo


# ---------------------------------------------------------------- packing
def _pack_nodes(deg):
    """Assign each node to a (block, slot) so that every block has at most
    NODE_CAP nodes and block in-degree sums are nearly equal (serpentine on
    sorted degree).  Returns (orig, inv): orig[slot] = node id or -1,
    inv[node] = slot."""
    order = np.argsort(-deg, kind="stable")
    n = order.size
    rows = np.arange(n) // N_BINS
    cols = np.arange(n) % N_BINS
    cols = np.where(rows % 2 == 0, cols, N_BINS - 1 - cols)
    slot = cols * NODE_CAP + rows
    orig = np.full(SLOTS, -1, dtype=np.int64)
    orig[slot] = order
    inv = np.empty(n, dtype=np.int64)
    inv[order] = slot
    return orig, inv


def _preprocess(x, edge_attr, receivers):
    """Build per-core device arrays.  Returns (in_maps, orig, t_max)."""
    import ml_dtypes

    bf16 = ml_dtypes.bfloat16
    deg = np.bincount(receivers, minlength=N_NODES)
    orig, inv = _pack_nodes(deg)

    eslot = inv[receivers]                  # node slot of each edge's receiver
    blk = eslot // NODE_CAP                 # block id per edge
    lidx = (eslot % NODE_CAP).astype(np.float32)

    counts = np.bincount(blk, minlength=N_BINS)
    t_max = max(4, int(-(-counts.max() // 128)))   # tiles (of 128 edges) per block

    order_e = np.argsort(blk, kind="stable")
    blk_s = blk[order_e]
    starts = np.zeros(N_BINS, dtype=np.int64)
    np.cumsum(counts[:-1], out=starts[1:])
    pos = np.arange(N_EDGES) - starts[blk_s]       # position within block
    k = pos // 128
    p = pos % 128

    core = blk_s // BLOCKS_PER_CORE
    gblk = blk_s % BLOCKS_PER_CORE
    lidx_s = lidx[order_e]

    hi, lo = _split(edge_attr[order_e], bf16)

    # padded node features: x as bf16 (hi, lo), feature-major
    xt_full = np.zeros((SLOTS, D), dtype=np.float32)
    valid = orig >= 0
    xt_full[valid] = x[orig[valid]]
    xh_full, xl_full = _split(xt_full, bf16)

    n_tiles = BLOCKS_PER_CORE * t_max
    G = BLOCKS_PER_CORE // GROUP_BLOCKS
    in_maps = []
    for c in range(N_CORES):
        sel = core == c
        # layout: [G, 128 partitions, block-in-group, tile, {hi,lo}, 64]
        # -> each partition's slice of a group DMA is 8KB contiguous, and
        #    tile t's 128 columns are the fused [hi | lo] stationary.
        pay = np.zeros((G, 128, GROUP_BLOCKS, t_max, 2, D), dtype=bf16)
        g_ = gblk[sel] // GROUP_BLOCKS
        b_ = gblk[sel] % GROUP_BLOCKS
        pay[g_, p[sel], b_, k[sel], 0, :] = hi[sel]
        pay[g_, p[sel], b_, k[sel], 1, :] = lo[sel]
        la = np.full((128, n_tiles), float(NODE_CAP), dtype=bf16)
        la[p[sel], gblk[sel] * t_max + k[sel]] = lidx_s[sel]
        sl = slice(c * SLOTS_PER_CORE, (c + 1) * SLOTS_PER_CORE)
        xh = np.ascontiguousarray(xh_full[sl].T)             # [64, S] bf16
        xl = np.ascontiguousarray(xl_full[sl].T)             # [64, S] bf16
        in_maps.append(
            {
                "edges": pay.reshape(G * 128, GROUP_BLOCKS * t_max * 2 * D),
                "lidx": la,
                "xc": np.ascontiguousarray(np.vstack([xl, xh])),  # [128, S]
            }
        )
    return in_maps, orig, t_max


# ---------------------------------------------------------------- program
def _build_program(t_max):
    from contextlib import ExitStack

    import concourse.bacc as bacc
    import concourse.tile as tile
    from concourse import mybir

    f32 = mybir.dt.float32
    bf16 = mybir.dt.bfloat16
    G = BLOCKS_PER_CORE // GROUP_BLOCKS      # MLP groups per core (50)
    TPG = GROUP_BLOCKS * t_max               # edge tiles per group
    NPG = GROUP_BLOCKS * NODE_CAP            # nodes per group (256)
    NT = BLOCKS_PER_CORE * t_max             # edge tiles per core
    S = SLOTS_PER_CORE
    RELU = mybir.ActivationFunctionType.Relu

    nc = bacc.Bacc("TRN2", target_bir_lowering=False, debug=False)
    edges = nc.dram_tensor(
        "edges", [G * 128, TPG * 2 * D], bf16, kind="ExternalInput"
    ).ap()
    lidx = nc.dram_tensor("lidx", [128, NT], bf16, kind="ExternalInput").ap()
    xc = nc.dram_tensor("xc", [128, S], bf16, kind="ExternalInput").ap()
    w1xs = nc.dram_tensor("w1xs", [128, D_HID], bf16, kind="ExternalInput").ap()
    # W1x_hi lives in partitions 64:128 so it can contract directly against
    # the x_hi half of xc (same base partition); rows 0:64 are zero padding.
    w1xh = nc.dram_tensor("w1xh", [128, D_HID], bf16, kind="ExternalInput").ap()
    w1aa = nc.dram_tensor("w1aa", [128, D_HID], f32, kind="ExternalInput").ap()
    w2h = nc.dram_tensor("w2h", [128, 128], bf16, kind="ExternalInput").ap()
    w2l = nc.dram_tensor("w2l", [128, 128], bf16, kind="ExternalInput").ap()
    b1 = nc.dram_tensor("b1", [128, 2], f32, kind="ExternalInput").ap()
    b2 = nc.dram_tensor("b2", [D, 1], f32, kind="ExternalInput").ap()
    out = nc.dram_tensor("out_t", [D, S], f32, kind="ExternalOutput").ap()

    with tile.TileContext(nc) as tc, ExitStack() as ctx:
        const = ctx.enter_context(tc.tile_pool(name="const", bufs=1))
        epool = ctx.enter_context(tc.tile_pool(name="epool", bufs=4))
        ohpool = ctx.enter_context(tc.tile_pool(name="ohpool", bufs=3))
        spool = ctx.enter_context(tc.tile_pool(name="spool", bufs=3))
        hpool = ctx.enter_context(tc.tile_pool(name="hpool", bufs=4))
        ps_a = ctx.enter_context(tc.tile_pool(name="ps_a", bufs=2, space="PSUM"))
        ps_h = ctx.enter_context(tc.tile_pool(name="ps_h", bufs=4, space="PSUM"))
        ps_o = ctx.enter_context(tc.tile_pool(name="ps_o", bufs=2, space="PSUM"))

        iota_i = const.tile([128, TPG * NODE_CAP], mybir.dt.int32)
        nc.gpsimd.iota(
            iota_i[:], pattern=[[0, TPG], [1, NODE_CAP]], channel_multiplier=0
        )
        iota_b = const.tile([128, TPG * NODE_CAP], bf16)
        nc.vector.tensor_copy(iota_b[:], iota_i[:])

        xc_sb = const.tile([128, S], bf16)
        nc.sync.dma_start(xc_sb[:], xc[:])
        lidx_sb = const.tile([128, NT], bf16)
        nc.sync.dma_start(lidx_sb[:], lidx[:])
        w1xs_sb = const.tile([128, D_HID], bf16)
        nc.sync.dma_start(w1xs_sb[:], w1xs[:])
        w1xh_sb = const.tile([128, D_HID], bf16)
        nc.sync.dma_start(w1xh_sb[:], w1xh[:])
        w1aa_sb = const.tile([128, D_HID], f32)
        nc.sync.dma_start(w1aa_sb[:], w1aa[:])
        w2h_sb = const.tile([128, 128], bf16)
        nc.sync.dma_start(w2h_sb[:], w2h[:])
        w2l_sb = const.tile([128, 128], bf16)
        nc.sync.dma_start(w2l_sb[:], w2l[:])
        b1_sb = const.tile([128, 2], f32)
        nc.sync.dma_start(b1_sb[:], b1[:])
        b2_sb = const.tile([D, 1], f32)
        nc.sync.dma_start(b2_sb[:], b2[:])

        # output staged in chunks so the store DMA streams during compute
        OCHUNK = 5                                    # groups per out chunk
        out_tiles = [
            const.tile([D, OCHUNK * NPG], f32, name=f"outc{i}", tag=f"out{i}")
            for i in range((G + OCHUNK - 1) // OCHUNK)
        ]

        def scatter_stage(g):
            """DMA + one-hot + 32 scatter matmuls -> (agg PSUM, sums SBUF)."""
            ech = epool.tile([128, TPG * 2 * D], bf16)
            nc.sync.dma_start(ech[:], edges[g * 128 : (g + 1) * 128, :])
            oh = ohpool.tile([128, TPG * NODE_CAP], bf16)
            nc.vector.tensor_tensor(
                out=oh[:].rearrange("p (t l) -> p t l", l=NODE_CAP),
                in0=iota_b[:].rearrange("p (t l) -> p t l", l=NODE_CAP),
                in1=lidx_sb[:, g * TPG : (g + 1) * TPG, None].to_broadcast(
                    [128, TPG, NODE_CAP]
                ),
                op=mybir.AluOpType.is_equal,
            )
            # one matmul per 128-edge tile with the fused [hi|lo] stationary
            # -> PSUM rows 0:64 = sum(hi), rows 64:128 = sum(lo)
            agg_ps = ps_a.tile([128, NPG], f32)
            for b in range(GROUP_BLOCKS):
                for kk in range(t_max):
                    t = b * t_max + kk
                    nc.tensor.matmul(
                        out=agg_ps[:, b * NODE_CAP : (b + 1) * NODE_CAP],
                        lhsT=ech[:, t * 2 * D : (t + 1) * 2 * D],
                        rhs=oh[:, t * NODE_CAP : (t + 1) * NODE_CAP],
                        start=(kk == 0),
                        stop=(kk == t_max - 1),
                    )
            sums_sb = spool.tile([128, NPG], f32)
            nc.vector.tensor_copy(sums_sb[:], agg_ps[:])
            return sums_sb

        def h_stage(g, sums_sb):
            """6 matmuls + relu + bf16 (hi, lo) split of h."""
            gs = slice(g * NPG, (g + 1) * NPG)
            his, los = [], []
            for hh in range(2):
                hsl = slice(hh * 128, (hh + 1) * 128)
                h_ps = ps_h.tile([128, NPG], f32)
                # cross terms: [W1x_hi; W1x_lo] against [x_lo; x_hi]
                nc.tensor.matmul(
                    out=h_ps[:], lhsT=w1xs_sb[:, hsl], rhs=xc_sb[:, gs],
                    start=True, stop=False,
                )
                # main term: W1x_hi @ x_hi (both at base partition 64)
                nc.tensor.matmul(
                    out=h_ps[:], lhsT=w1xh_sb[D:, hsl], rhs=xc_sb[D:, gs],
                    start=False, stop=False,
                )
                # agg term (fp32, exact): [W1a; W1a] @ [sum_hi; sum_lo]
                nc.tensor.matmul(
                    out=h_ps[:], lhsT=w1aa_sb[:, hsl], rhs=sums_sb[:],
                    start=False, stop=True,
                )
                # h_f (fp32, ACT) and h_hi (bf16, DVE) both read the PSUM
                # independently -> no serial dependency between them.
                h_f = hpool.tile([128, NPG], f32, tag="h_f")
                nc.scalar.activation(
                    h_f[:], h_ps[:], RELU, bias=b1_sb[:, hh : hh + 1], scale=1.0
                )
                h_hi = hpool.tile([128, NPG], bf16, tag="h_hi")
                nc.vector.tensor_scalar(
                    out=h_hi[:],
                    in0=h_ps[:],
                    scalar1=b1_sb[:, hh : hh + 1],
                    scalar2=0.0,
                    op0=mybir.AluOpType.add,
                    op1=mybir.AluOpType.max,
                )
                h_lo = hpool.tile([128, NPG], bf16, tag="h_lo")
                nc.gpsimd.tensor_tensor(
                    out=h_lo[:], in0=h_f[:], in1=h_hi[:],
                    op=mybir.AluOpType.subtract,
                )
                his.append(h_hi)
                los.append(h_lo)
            return his, los

        def out_stage(g, his, los):
            """6 bf16 matmuls + bias; stream the chunk DMA when complete."""
            o_ps = ps_o.tile([D, NPG], f32)
            mms = []
            for hh in range(2):
                csl = slice(hh * D, (hh + 1) * D)
                mms += [
                    (w2h_sb[:, csl], his[hh][:]),
                    (w2h_sb[:, csl], los[hh][:]),
                    (w2l_sb[:, csl], his[hh][:]),
                ]
            for i, (lt, rt) in enumerate(mms):
                nc.tensor.matmul(
                    out=o_ps[:], lhsT=lt, rhs=rt,
                    start=(i == 0), stop=(i == len(mms) - 1),
                )
            ci, co = g // OCHUNK, g % OCHUNK
            otile = out_tiles[ci]
            # identity + per-partition bias add on ACT
            nc.scalar.activation(
                otile[:, co * NPG : (co + 1) * NPG],
                o_ps[:],
                mybir.ActivationFunctionType.Identity,
                bias=b2_sb[:, 0:1],
                scale=1.0,
            )
            lastco = min(OCHUNK, G - ci * OCHUNK) - 1
            if co == lastco:
                nc.sync.dma_start(
                    out[:, ci * OCHUNK * NPG : ci * OCHUNK * NPG + (lastco + 1) * NPG],
                    otile[:, : (lastco + 1) * NPG],
                )

        # software pipeline: each group's out-stage is emitted after the next
        # group's scatter, so the PE has independent work while the
        # relu/split chain (ACT -> DVE -> GpSimd) completes.
        pend = None
        for g in range(G):
            sums_sb = scatter_stage(g)
            if pend is not None:
                out_stage(*pend)
            pend = (g, *h_stage(g, sums_sb))
        out_stage(*pend)

    nc.compile()
    return nc


def _get_program(t_max):
    if t_max not in _cache:
        _cache[t_max] = _build_program(t_max)
    return _cache[t_max]


def _weight_maps(W1, b1, W2, b2):
    import ml_dtypes

    bf16 = ml_dtypes.bfloat16
    w1x = np.ascontiguousarray(W1[0:D, :])
    w1a = np.ascontiguousarray(W1[D : 2 * D, :])
    w1x_hi, w1x_lo = _split(w1x, bf16)
    w2p = np.concatenate([W2[0:128, :], W2[128:256, :]], axis=1)  # [128, 128]
    w2_hi, w2_lo = _split(w2p, bf16)
    return {
        "w1xs": np.ascontiguousarray(np.vstack([w1x_hi, w1x_lo])),
        "w1xh": np.ascontiguousarray(
            np.vstack([np.zeros_like(w1x_hi), w1x_hi])
        ),
        "w1aa": np.ascontiguousarray(np.vstack([w1a, w1a])),
        "w2h": np.ascontiguousarray(w2_hi),
        "w2l": np.ascontiguousarray(w2_lo),
        "b1": np.ascontiguousarray(b1.reshape(2, 128).T),
        "b2": np.ascontiguousarray(b2.reshape(D, 1)),
    }


# ---------------------------------------------------------------- entry
def kernel(x, edge_attr, edge_index, pos, W1, b1, W2, b2, _trace=False, _tmpdir=None):
    from concourse.bass_utils import run_bass_kernel_spmd

    x = np.asarray(x, dtype=np.float32)
    edge_attr = np.asarray(edge_attr, dtype=np.float32)
    receivers = np.asarray(edge_index[1]).astype(np.int64)
    W1 = np.asarray(W1, dtype=np.float32)
    b1 = np.asarray(b1, dtype=np.float32)
    W2 = np.asarray(W2, dtype=np.float32)
    b2 = np.asarray(b2, dtype=np.float32)

    in_maps, orig, t_max = _preprocess(x, edge_attr, receivers)
    wmap = _weight_maps(W1, b1, W2, b2)
    for m in in_maps:
        m.update(wmap)

    nc = _get_program(t_max)
    res = run_bass_kernel_spmd(
        nc, in_maps, list(range(N_CORES)), trace=_trace, tmpdir=_tmpdir
    )

    big = np.concatenate([r["out_t"] for r in res.results], axis=1)  # [64, SLOTS]
    valid = orig >= 0
    result = np.empty((N_NODES, D), dtype=np.float32)
    result[orig[valid]] = big.T[valid]
    if _trace:
        kernel.last_results = res
    return result


# revision 31
# speedup vs baseline: 1.3440x; 1.0155x over previous
"""Trainium2 Bass kernel for nn_NodeBlock (GNN message passing).

Computes, for a graph with N=100000 nodes and E=1600000 edges:
    agg = segment_sum(edge_attr, edge_index[1], N)        # [N, 64]
    h   = relu(concat([x, agg], 1) @ W1 + b1)             # [N, 256]
    out = h @ W2 + b2                                     # [N, 64]

Strategy (8 NeuronCores, no collectives):
  * Nodes are packed into blocks of <= NODE_CAP=32 nodes such that every
    block receives <= T_MAX*128 = 512 edges (host-side serpentine packing
    on in-degree).  Core c owns blocks [c*400, (c+1)*400).
  * Edges are bucketed by receiver block on the host, so each core's
    shard holds exactly the edges targeting its own nodes: no all-reduce.
  * fp32 values are shipped as bf16 (hi, lo) pairs (hi = bf16(v),
    lo = bf16(v - hi)); matmuls run at bf16 speed and all accumulation is
    fp32 in PSUM, keeping ~2^-18 relative precision.
  * Scatter-add: each 128-edge tile becomes a one-hot [128, 32] matrix
    (iota vs local-node-index compare, built 32 tiles per vector op) and
    a single matmul with the fused [hi|lo] stationary accumulates
    [hi-feat; lo-feat] partial aggregates [128, 32] in PSUM.  The hi/lo
    fold happens for free inside the MLP's agg matmul, whose fp32 weight
    is W1a stacked twice along the contraction dim.
  * The MLP runs on 256-node groups in feature-major layout (nodes on
    the free dim); the x-side and W2-side matmuls use bf16 (hi, lo)
    3-term products, with cross terms fused by stacking [lo; hi] inputs
    against [hi; lo] weights along the contraction dim.
"""

import sys

sys.path.insert(0, "/opt/trn_rl_repo")

import numpy as np

# ---------------------------------------------------------------- constants
N_NODES = 100000
N_EDGES = 1600000
D = 64            # d_node == d_edge == d_out
D_HID = 256
N_CORES = 8

NODE_CAP = 32         # node slots per block
N_BINS = 3200         # total blocks (divisible by N_CORES)
BLOCKS_PER_CORE = N_BINS // N_CORES          # 400
GROUP_BLOCKS = 16                            # blocks per MLP group
SLOTS = N_BINS * NODE_CAP                    # 102400 padded node slots
SLOTS_PER_CORE = SLOTS // N_CORES            # 12800

_cache = {}


def _split(a, bf16):
    """Return (hi, lo) bf16 pair with hi + lo ~= a (to ~2^-18 rel)."""
    hi = a.astype(bf16)
    lo = (a - hi.astype(np.float32)).astype(bf16)
    return hi, lo


# ---------------------------------------------------------------- packing
def _pack_nodes(deg):
    """Assign each node to a (block, slot) so that every block has at most
    NODE_CAP nodes and block in-degree sums are nearly equal (serpentine on
    sorted degree).  Returns (orig, inv): orig[slot] = node id or -1,
    inv[node] = slot."""
    order = np.argsort(-deg, kind="stable")
    n = order.size
    rows = np.arange(n) // N_BINS
    cols = np.arange(n) % N_BINS
    cols = np.where(rows % 2 == 0, cols, N_BINS - 1 - cols)
    slot = cols * NODE_CAP + rows
    orig = np.full(SLOTS, -1, dtype=np.int64)
    orig[slot] = order
    inv = np.empty(n, dtype=np.int64)
    inv[order] = slot
    return orig, inv


def _preprocess(x, edge_attr, receivers):
    """Build per-core device arrays.  Returns (in_maps, orig, t_max)."""
    import ml_dtypes

    bf16 = ml_dtypes.bfloat16
    deg = np.bincount(receivers, minlength=N_NODES)
    orig, inv = _pack_nodes(deg)

    eslot = inv[receivers]                  # node slot of each edge's receiver
    blk = eslot // NODE_CAP                 # block id per edge
    lidx = (eslot % NODE_CAP).astype(np.float32)

    counts = np.bincount(blk, minlength=N_BINS)
    t_max = max(4, int(-(-counts.max() // 128)))   # tiles (of 128 edges) per block

    order_e = np.argsort(blk, kind="stable")
    blk_s = blk[order_e]
    starts = np.zeros(N_BINS, dtype=np.int64)
    np.cumsum(counts[:-1], out=starts[1:])
    pos = np.arange(N_EDGES) - starts[blk_s]       # position within block
    k = pos // 128
    p = pos % 128

    core = blk_s // BLOCKS_PER_CORE
    gblk = blk_s % BLOCKS_PER_CORE
    lidx_s = lidx[order_e]

    hi, lo = _split(edge_attr[order_e], bf16)

    # padded node features: x as bf16 (hi, lo), feature-major
    xt_full = np.zeros((SLOTS, D), dtype=np.float32)
    valid = orig >= 0
    xt_full[valid] = x[orig[valid]]
    xh_full, xl_full = _split(xt_full, bf16)

    n_tiles = BLOCKS_PER_CORE * t_max
    G = BLOCKS_PER_CORE // GROUP_BLOCKS
    in_maps = []
    for c in range(N_CORES):
        sel = core == c
        # layout: [G, 128 partitions, block-in-group, tile, {hi,lo}, 64]
        # -> each partition's slice of a group DMA is 8KB contiguous, and
        #    tile t's 128 columns are the fused [hi | lo] stationary.
        pay = np.zeros((G, 128, GROUP_BLOCKS, t_max, 2, D), dtype=bf16)
        g_ = gblk[sel] // GROUP_BLOCKS
        b_ = gblk[sel] % GROUP_BLOCKS
        pay[g_, p[sel], b_, k[sel], 0, :] = hi[sel]
        pay[g_, p[sel], b_, k[sel], 1, :] = lo[sel]
        la = np.full((128, n_tiles), float(NODE_CAP), dtype=bf16)
        la[p[sel], gblk[sel] * t_max + k[sel]] = lidx_s[sel]
        sl = slice(c * SLOTS_PER_CORE, (c + 1) * SLOTS_PER_CORE)
        xh = np.ascontiguousarray(xh_full[sl].T)             # [64, S] bf16
        xl = np.ascontiguousarray(xl_full[sl].T)             # [64, S] bf16
        in_maps.append(
            {
                "edges": pay.reshape(G * 128, GROUP_BLOCKS * t_max * 2 * D),
                "lidx": la,
                "xc": np.ascontiguousarray(np.vstack([xl, xh])),  # [128, S]
            }
        )
    return in_maps, orig, t_max


# ---------------------------------------------------------------- program
def _build_program(t_max):
    from contextlib import ExitStack

    import concourse.bacc as bacc
    import concourse.tile as tile
    from concourse import mybir

    f32 = mybir.dt.float32
    bf16 = mybir.dt.bfloat16
    G = BLOCKS_PER_CORE // GROUP_BLOCKS      # MLP groups per core (50)
    TPG = GROUP_BLOCKS * t_max               # edge tiles per group
    NPG = GROUP_BLOCKS * NODE_CAP            # nodes per group (256)
    NT = BLOCKS_PER_CORE * t_max             # edge tiles per core
    S = SLOTS_PER_CORE
    RELU = mybir.ActivationFunctionType.Relu

    nc = bacc.Bacc("TRN2", target_bir_lowering=False, debug=False)
    edges = nc.dram_tensor(
        "edges", [G * 128, TPG * 2 * D], bf16, kind="ExternalInput"
    ).ap()
    lidx = nc.dram_tensor("lidx", [128, NT], bf16, kind="ExternalInput").ap()
    xc = nc.dram_tensor("xc", [128, S], bf16, kind="ExternalInput").ap()
    w1xs = nc.dram_tensor("w1xs", [128, D_HID], bf16, kind="ExternalInput").ap()
    # W1x_hi lives in partitions 64:128 so it can contract directly against
    # the x_hi half of xc (same base partition); rows 0:64 are zero padding.
    w1xh = nc.dram_tensor("w1xh", [128, D_HID], bf16, kind="ExternalInput").ap()
    w1aa = nc.dram_tensor("w1aa", [128, D_HID], f32, kind="ExternalInput").ap()
    w2h = nc.dram_tensor("w2h", [128, 128], bf16, kind="ExternalInput").ap()
    w2l = nc.dram_tensor("w2l", [128, 128], bf16, kind="ExternalInput").ap()
    b1 = nc.dram_tensor("b1", [128, 2], f32, kind="ExternalInput").ap()
    b2 = nc.dram_tensor("b2", [D, 1], f32, kind="ExternalInput").ap()
    out = nc.dram_tensor("out_t", [D, S], f32, kind="ExternalOutput").ap()

    with tile.TileContext(nc) as tc, ExitStack() as ctx:
        const = ctx.enter_context(tc.tile_pool(name="const", bufs=1))
        epool = ctx.enter_context(tc.tile_pool(name="epool", bufs=4))
        ohpool = ctx.enter_context(tc.tile_pool(name="ohpool", bufs=3))
        spool = ctx.enter_context(tc.tile_pool(name="spool", bufs=3))
        hpool = ctx.enter_context(tc.tile_pool(name="hpool", bufs=4))
        ps_a = ctx.enter_context(tc.tile_pool(name="ps_a", bufs=2, space="PSUM"))
        ps_h = ctx.enter_context(tc.tile_pool(name="ps_h", bufs=4, space="PSUM"))
        ps_o = ctx.enter_context(tc.tile_pool(name="ps_o", bufs=2, space="PSUM"))

        iota_i = const.tile([128, TPG * NODE_CAP], mybir.dt.int32)
        nc.gpsimd.iota(
            iota_i[:], pattern=[[0, TPG], [1, NODE_CAP]], channel_multiplier=0
        )
        iota_b = const.tile([128, TPG * NODE_CAP], bf16)
        nc.vector.tensor_copy(iota_b[:], iota_i[:])

        xc_sb = const.tile([128, S], bf16)
        nc.sync.dma_start(xc_sb[:], xc[:])
        lidx_sb = const.tile([128, NT], bf16)
        nc.sync.dma_start(lidx_sb[:], lidx[:])
        w1xs_sb = const.tile([128, D_HID], bf16)
        nc.sync.dma_start(w1xs_sb[:], w1xs[:])
        w1xh_sb = const.tile([128, D_HID], bf16)
        nc.sync.dma_start(w1xh_sb[:], w1xh[:])
        w1aa_sb = const.tile([128, D_HID], f32)
        nc.sync.dma_start(w1aa_sb[:], w1aa[:])
        w2h_sb = const.tile([128, 128], bf16)
        nc.sync.dma_start(w2h_sb[:], w2h[:])
        w2l_sb = const.tile([128, 128], bf16)
        nc.sync.dma_start(w2l_sb[:], w2l[:])
        b1_sb = const.tile([128, 2], f32)
        nc.sync.dma_start(b1_sb[:], b1[:])
        b2_sb = const.tile([D, 1], f32)
        nc.sync.dma_start(b2_sb[:], b2[:])

        # output staged in chunks so the store DMA streams during compute
        OCHUNK = 5                                    # groups per out chunk
        out_tiles = [
            const.tile([D, OCHUNK * NPG], f32, name=f"outc{i}", tag=f"out{i}")
            for i in range((G + OCHUNK - 1) // OCHUNK)
        ]

        def scatter_stage(g):
            """DMA + one-hot + 32 scatter matmuls -> (agg PSUM, sums SBUF)."""
            ech = epool.tile([128, TPG * 2 * D], bf16)
            nc.sync.dma_start(ech[:], edges[g * 128 : (g + 1) * 128, :])
            oh = ohpool.tile([128, TPG * NODE_CAP], bf16)
            nc.vector.tensor_tensor(
                out=oh[:].rearrange("p (t l) -> p t l", l=NODE_CAP),
                in0=iota_b[:].rearrange("p (t l) -> p t l", l=NODE_CAP),
                in1=lidx_sb[:, g * TPG : (g + 1) * TPG, None].to_broadcast(
                    [128, TPG, NODE_CAP]
                ),
                op=mybir.AluOpType.is_equal,
            )
            # one matmul per 128-edge tile with the fused [hi|lo] stationary
            # -> PSUM rows 0:64 = sum(hi), rows 64:128 = sum(lo)
            agg_ps = ps_a.tile([128, NPG], f32)
            for b in range(GROUP_BLOCKS):
                for kk in range(t_max):
                    t = b * t_max + kk
                    nc.tensor.matmul(
                        out=agg_ps[:, b * NODE_CAP : (b + 1) * NODE_CAP],
                        lhsT=ech[:, t * 2 * D : (t + 1) * 2 * D],
                        rhs=oh[:, t * NODE_CAP : (t + 1) * NODE_CAP],
                        start=(kk == 0),
                        stop=(kk == t_max - 1),
                    )
            sums_sb = spool.tile([128, NPG], f32)
            nc.vector.tensor_copy(sums_sb[:], agg_ps[:])
            return sums_sb

        def hx_stage(g):
            """The x-side h matmuls; independent of the scatter result."""
            gs = slice(g * NPG, (g + 1) * NPG)
            h_pss = []
            for hh in range(2):
                hsl = slice(hh * 128, (hh + 1) * 128)
                h_ps = ps_h.tile([128, NPG], f32)
                # cross terms: [W1x_hi; W1x_lo] against [x_lo; x_hi]
                nc.tensor.matmul(
                    out=h_ps[:], lhsT=w1xs_sb[:, hsl], rhs=xc_sb[:, gs],
                    start=True, stop=False,
                )
                # main term: W1x_hi @ x_hi (both at base partition 64)
                nc.tensor.matmul(
                    out=h_ps[:], lhsT=w1xh_sb[D:, hsl], rhs=xc_sb[D:, gs],
                    start=False, stop=False,
                )
                h_pss.append(h_ps)
            return h_pss

        def h_stage(g, sums_sb, h_pss):
            """Finish h accumulation (agg term) + relu + bf16 (hi, lo) split."""
            his, los = [], []
            for hh in range(2):
                hsl = slice(hh * 128, (hh + 1) * 128)
                h_ps = h_pss[hh]
                # agg term (fp32, exact): [W1a; W1a] @ [sum_hi; sum_lo]
                nc.tensor.matmul(
                    out=h_ps[:], lhsT=w1aa_sb[:, hsl], rhs=sums_sb[:],
                    start=False, stop=True,
                )
                # h_f (fp32, ACT) and h_hi (bf16, DVE) both read the PSUM
                # independently -> no serial dependency between them.
                h_f = hpool.tile([128, NPG], f32, tag="h_f")
                nc.scalar.activation(
                    h_f[:], h_ps[:], RELU, bias=b1_sb[:, hh : hh + 1], scale=1.0
                )
                h_hi = hpool.tile([128, NPG], bf16, tag="h_hi")
                nc.vector.tensor_scalar(
                    out=h_hi[:],
                    in0=h_ps[:],
                    scalar1=b1_sb[:, hh : hh + 1],
                    scalar2=0.0,
                    op0=mybir.AluOpType.add,
                    op1=mybir.AluOpType.max,
                )
                h_lo = hpool.tile([128, NPG], bf16, tag="h_lo")
                nc.gpsimd.tensor_tensor(
                    out=h_lo[:], in0=h_f[:], in1=h_hi[:],
                    op=mybir.AluOpType.subtract,
                )
                his.append(h_hi)
                los.append(h_lo)
            return his, los

        def out_stage(g, his, los):
            """6 bf16 matmuls + bias; stream the chunk DMA when complete."""
            o_ps = ps_o.tile([D, NPG], f32)
            # h_lo-dependent matmuls last: h_lo is the tail of the split chain
            mms = []
            for hh in range(2):
                csl = slice(hh * D, (hh + 1) * D)
                mms += [
                    (w2h_sb[:, csl], his[hh][:]),
                    (w2l_sb[:, csl], his[hh][:]),
                ]
            for hh in range(2):
                csl = slice(hh * D, (hh + 1) * D)
                mms.append((w2h_sb[:, csl], los[hh][:]))
            for i, (lt, rt) in enumerate(mms):
                nc.tensor.matmul(
                    out=o_ps[:], lhsT=lt, rhs=rt,
                    start=(i == 0), stop=(i == len(mms) - 1),
                )
            ci, co = g // OCHUNK, g % OCHUNK
            otile = out_tiles[ci]
            # identity + per-partition bias add on ACT
            nc.scalar.activation(
                otile[:, co * NPG : (co + 1) * NPG],
                o_ps[:],
                mybir.ActivationFunctionType.Identity,
                bias=b2_sb[:, 0:1],
                scale=1.0,
            )
            lastco = min(OCHUNK, G - ci * OCHUNK) - 1
            if co == lastco:
                nc.sync.dma_start(
                    out[:, ci * OCHUNK * NPG : ci * OCHUNK * NPG + (lastco + 1) * NPG],
                    otile[:, : (lastco + 1) * NPG],
                )

        # software pipeline: each group's out-stage is emitted after the next
        # group's scatter + x-side matmuls, so the PE has independent work
        # while the relu/split chain (ACT/DVE -> GpSimd) completes.
        pend = None
        for g in range(G):
            sums_sb = scatter_stage(g)
            h_pss = hx_stage(g)
            if pend is not None:
                out_stage(*pend)
            pend = (g, *h_stage(g, sums_sb, h_pss))
        out_stage(*pend)

    nc.compile()
    return nc


def _get_program(t_max):
    if t_max not in _cache:
        _cache[t_max] = _build_program(t_max)
    return _cache[t_max]


def _weight_maps(W1, b1, W2, b2):
    import ml_dtypes

    bf16 = ml_dtypes.bfloat16
    w1x = np.ascontiguousarray(W1[0:D, :])
    w1a = np.ascontiguousarray(W1[D : 2 * D, :])
    w1x_hi, w1x_lo = _split(w1x, bf16)
    w2p = np.concatenate([W2[0:128, :], W2[128:256, :]], axis=1)  # [128, 128]
    w2_hi, w2_lo = _split(w2p, bf16)
    return {
        "w1xs": np.ascontiguousarray(np.vstack([w1x_hi, w1x_lo])),
        "w1xh": np.ascontiguousarray(
            np.vstack([np.zeros_like(w1x_hi), w1x_hi])
        ),
        "w1aa": np.ascontiguousarray(np.vstack([w1a, w1a])),
        "w2h": np.ascontiguousarray(w2_hi),
        "w2l": np.ascontiguousarray(w2_lo),
        "b1": np.ascontiguousarray(b1.reshape(2, 128).T),
        "b2": np.ascontiguousarray(b2.reshape(D, 1)),
    }


# ---------------------------------------------------------------- entry
def kernel(x, edge_attr, edge_index, pos, W1, b1, W2, b2, _trace=False, _tmpdir=None):
    from concourse.bass_utils import run_bass_kernel_spmd

    x = np.asarray(x, dtype=np.float32)
    edge_attr = np.asarray(edge_attr, dtype=np.float32)
    receivers = np.asarray(edge_index[1]).astype(np.int64)
    W1 = np.asarray(W1, dtype=np.float32)
    b1 = np.asarray(b1, dtype=np.float32)
    W2 = np.asarray(W2, dtype=np.float32)
    b2 = np.asarray(b2, dtype=np.float32)

    in_maps, orig, t_max = _preprocess(x, edge_attr, receivers)
    wmap = _weight_maps(W1, b1, W2, b2)
    for m in in_maps:
        m.update(wmap)

    nc = _get_program(t_max)
    res = run_bass_kernel_spmd(
        nc, in_maps, list(range(N_CORES)), trace=_trace, tmpdir=_tmpdir
    )

    big = np.concatenate([r["out_t"] for r in res.results], axis=1)  # [64, SLOTS]
    valid = orig >= 0
    result = np.empty((N_NODES, D), dtype=np.float32)
    result[orig[valid]] = big.T[valid]
    if _trace:
        kernel.last_results = res
    return result
